# revision 7
# baseline (speedup 1.0000x reference)
"""Trainium2 Bass kernel for nn_CausalMask (gumbel-sigmoid node/edge masks +
symmetric scatter into a [P, P] edge mask), SPMD across 8 NeuronCores.

Strategy (row-sharded scatter):
  - Core k owns rows [k*768, (k+1)*768) of the [6144, 6144] edge mask.
    Its block lives in SBUF as [128 partitions x 36864 f32] (partition p
    holds mask rows 6p..6p+5 of the block, row-major).
  - The host routes each scattered entry (both (r,c) and (c,r) of every
    edge) to (core, partition, window, slot). A window is a 768-f32
    column segment of one row: 48 windows per partition.
  - On device: ACT/DVE compute the gumbel-sigmoid edge values for the
    padded entry buffer; 48 gpsimd local_scatter calls compose the block
    (zeros + values fused). f32 exactness via the int16-pair bitcast
    trick (value f32 bits land as two adjacent int16 scatters).
  - 8 large HWDGE DMAs stream the block to HBM, pipelined with the
    scatters (chunk j = windows 6j..6j+5 = one SBUF tile).
"""

import sys
import types

for _p in ("/opt/trn_rl_repo", "/root/.axon_site"):
    if _p not in sys.path:
        sys.path.insert(0, _p)

# NTFF profile hook (used only when BASS_TRACE=1): the image's antenv lacks
# axon_hooks, so provide it via sys.modules before bass_utils imports it.
if "antenv.axon_hooks" not in sys.modules:
    _m = types.ModuleType("antenv.axon_hooks")

    def _get_hook():
        try:
            from trn_agent_boot.trn_boot import _ntff_profile_via_ctypes

            return _ntff_profile_via_ctypes("/opt/axon/libaxon_pjrt.so")
        except Exception:
            return None

    _m.get_axon_ntff_profile_hook = _get_hook
    _m.set_axon_ntff_profile_hook = lambda h: None
    sys.modules["antenv.axon_hooks"] = _m

import numpy as np

P = 6144          # num_patches
E = 262144        # number of edges
NCORES = 8
RPB = P // NCORES     # 768 rows per core block
RPP = RPB // 128      # 6 mask rows per partition
NCT = 8               # column tiles per row
WF = P // NCT         # 768 f32 per window
W = RPP * NCT         # 48 windows per partition
CHW = RPP             # windows per DMA chunk (6)
NCH = W // CHW        # 8 output DMA chunks
CHF = CHW * WF        # 4608 f32 per chunk per partition
TAU = 1.0
EPS = 1e-10

_BUILD_CACHE: dict[int, object] = {}
LAST_RESULTS = None   # BassKernelResults of the most recent run (for test.py)


def _build_program(K: int):
    """Build + finalize the SPMD Bass program for per-cell slot count K."""
    import concourse.bacc as bacc
    import concourse.mybir as mybir
    import concourse.tile as tile

    f32 = mybir.dt.float32
    i16 = mybir.dt.int16
    AF = mybir.ActivationFunctionType
    ALU = mybir.AluOpType
    LK = W * K
    NPF = P // 128  # 48 node values per partition

    nc = bacc.Bacc()
    # register EPS as a const AP so activation(bias=EPS) resolves
    _ct = nc.alloc_sbuf_tensor(f"const-f32-eps", [128, 1], f32)
    nc.gpsimd.memset(_ct.ap(), EPS)
    nc.const_aps.aps[(f32, EPS)] = _ct.ap()
    nc.all_engine_barrier()

    el = nc.declare_dram_parameter("el", [128, LK], f32, isOutput=False)
    eu = nc.declare_dram_parameter("eu", [128, LK], f32, isOutput=False)
    ei = nc.declare_dram_parameter("ei", [128, 2 * LK], i16, isOutput=False)
    nl = nc.declare_dram_parameter("nl", [P], f32, isOutput=False)
    nu = nc.declare_dram_parameter("nu", [P], f32, isOutput=False)
    eb = nc.declare_dram_parameter("edge_block", [RPB, P], f32, isOutput=True)
    nm = nc.declare_dram_parameter("node_mask", [P], f32, isOutput=True)

    CK = CHW * K  # padded entries per partition per chunk

    with tile.TileContext(nc) as tc:
        with tc.tile_pool(name="sbuf", bufs=1) as pool:
            tel = pool.tile([128, LK], f32, tag="tel")
            teu = pool.tile([128, LK], f32, tag="teu")
            tei = pool.tile([128, 2 * LK], i16, tag="tei")

            # prefetch all entry chunks on the ACT HWDGE ring so they never
            # queue behind the (SP-ring) output stream
            for j in range(NCH):
                cs = slice(j * CK, (j + 1) * CK)
                cs2 = slice(2 * j * CK, 2 * (j + 1) * CK)
                nc.scalar.dma_start(tei[:, cs2], ei[:, cs2])
                nc.scalar.dma_start(tel[:, cs], el[:, cs])
                nc.scalar.dma_start(teu[:, cs], eu[:, cs])

            # ---- per-chunk: compute gumbel-sigmoid values, compose windows
            #      via local_scatter, stream chunk to HBM
            ebf = eb[:, :].rearrange("(p a) b -> p (a b)", p=128)  # [128, 36864]
            for j in range(NCH):
                cs = slice(j * CK, (j + 1) * CK)
                # g = -ln(-ln(u + eps) + eps);  v = sigmoid((logit + g) / tau)
                nc.scalar.activation(teu[:, cs], teu[:, cs], AF.Ln, bias=EPS)
                # guard: ln(u+eps) must stay <= 0 so -ln(..)+eps > 0
                nc.vector.tensor_scalar_min(teu[:, cs], teu[:, cs], 0.0)
                nc.scalar.activation(teu[:, cs], teu[:, cs], AF.Ln, bias=EPS, scale=-1.0)
                nc.vector.tensor_tensor(tel[:, cs], tel[:, cs], teu[:, cs], op=ALU.subtract)
                nc.scalar.activation(tel[:, cs], tel[:, cs], AF.Sigmoid, scale=1.0 / TAU)

                blk = pool.tile([128, CHF], f32, tag=f"blk{j}")
                for wi in range(CHW):
                    w = j * CHW + wi
                    nc.gpsimd.local_scatter(
                        out_ap=blk[:, wi * WF : (wi + 1) * WF].bitcast(i16),
                        data_ap=tel[:, w * K : (w + 1) * K].bitcast(i16),
                        idxs_ap=tei[:, w * 2 * K : (w + 1) * 2 * K],
                        channels=128,
                        num_elems=2 * WF,
                        num_idxs=2 * K,
                    )
                    # stream out per 2 windows: shorter pipeline tail and
                    # earlier DMA start than one DMA per 6-window chunk
                    if wi % 2 == 1:
                        lo = (j * CHW + wi - 1) * WF
                        hi = (j * CHW + wi + 1) * WF
                        nc.sync.dma_start(
                            ebf[:, lo:hi], blk[:, (wi - 1) * WF : (wi + 1) * WF]
                        )

            # ---- node mask (identical on every core; tiny — run at the end
            #      so it never delays the scatter pipeline)
            tnl = pool.tile([128, NPF], f32, tag="tnl")
            tnu = pool.tile([128, NPF], f32, tag="tnu")
            nc.scalar.dma_start(tnl[:], nl[:].rearrange("(a b) -> a b", a=128))
            nc.scalar.dma_start(tnu[:], nu[:].rearrange("(a b) -> a b", a=128))
            nc.scalar.activation(tnu[:], tnu[:], AF.Ln, bias=EPS)
            nc.vector.tensor_scalar_min(tnu[:], tnu[:], 0.0)
            nc.scalar.activation(tnu[:], tnu[:], AF.Ln, bias=EPS, scale=-1.0)
            nc.vector.tensor_tensor(tnl[:], tnl[:], tnu[:], op=ALU.subtract)
            nc.scalar.activation(tnl[:], tnl[:], AF.Sigmoid, scale=1.0 / TAU)
            nc.sync.dma_start(nm[:].rearrange("(a b) -> a b", a=128), tnl[:])

    nc.finalize()
    return nc


def _route_entries(rows: np.ndarray, cols: np.ndarray):
    """Route 2E scattered entries to (core, partition, window, slot).

    Returns (K, dest, order) where order indexes into the concatenated
    entry list (first E: (r,c), second E: (c,r)), dest is the flat slot
    index into the per-core padded buffers [NCORES, 128, W, K], and K the
    global max entries per (core, partition, window) cell.
    """
    rr = np.concatenate([rows, cols]).astype(np.int64)
    cc = np.concatenate([cols, rows]).astype(np.int64)

    core = rr // RPB
    lr = rr - core * RPB
    p = lr // RPP
    q = lr - p * RPP
    ct = cc // WF
    cpos = cc - ct * WF
    w = q * NCT + ct
    cell = (core * 128 + p) * W + w

    order = np.argsort(cell, kind="stable")
    cell_s = cell[order]
    # rank within equal-cell runs
    first = np.r_[0, np.flatnonzero(np.diff(cell_s)) + 1]
    counts = np.diff(np.r_[first, len(cell_s)])
    K = int(counts.max())
    slot = np.arange(len(cell_s), dtype=np.int64) - np.repeat(first, counts)
    dest = cell_s * K + slot
    return K, dest, order, cpos


def kernel(node_logits, edge_logits, u_node, u_edge, rows, cols):
    global LAST_RESULTS
    from concourse.bass_utils import run_bass_kernel_spmd

    node_logits = np.asarray(node_logits, np.float32)
    edge_logits = np.asarray(edge_logits, np.float32)
    u_node = np.asarray(u_node, np.float32)
    u_edge = np.asarray(u_edge, np.float32)
    rows = np.asarray(rows)
    cols = np.asarray(cols)

    K, dest, order, cpos = _route_entries(rows, cols)

    nc = _BUILD_CACHE.get(K)
    if nc is None:
        nc = _build_program(K)
        _BUILD_CACHE[K] = nc

    # padded per-core buffers (padding values never scattered: idx = -1;
    # u=0 padding is safe through the clamped log-log pipeline)
    ncell = NCORES * 128 * W
    el_pad = np.zeros(ncell * K, np.float32)
    eu_pad = np.zeros(ncell * K, np.float32)
    ei_pad = np.full(ncell * 2 * K, -1, np.int16)

    ee = np.concatenate([np.arange(E), np.arange(E)])[order]
    el_pad[dest] = edge_logits[ee]
    eu_pad[dest] = u_edge[ee]
    cpos_s = cpos[order]
    ei_pad[2 * dest] = (2 * cpos_s).astype(np.int16)
    ei_pad[2 * dest + 1] = (2 * cpos_s + 1).astype(np.int16)

    el_pad = el_pad.reshape(NCORES, 128, W * K)
    eu_pad = eu_pad.reshape(NCORES, 128, W * K)
    ei_pad = ei_pad.reshape(NCORES, 128, 2 * W * K)

    in_maps = [
        {
            "el": el_pad[c],
            "eu": eu_pad[c],
            "ei": ei_pad[c],
            "nl": node_logits,
            "nu": u_node,
        }
        for c in range(NCORES)
    ]

    res = run_bass_kernel_spmd(nc, in_maps, list(range(NCORES)))
    LAST_RESULTS = res

    edge_mask = np.concatenate(
        [res.results[c]["edge_block"] for c in range(NCORES)], axis=0
    )
    node_mask = res.results[0]["node_mask"]
    return node_mask, edge_mask


# revision 9
# speedup vs baseline: 1.1083x; 1.1083x over previous
"""Trainium2 Bass kernel for nn_CausalMask (gumbel-sigmoid node/edge masks +
symmetric scatter into a [P, P] edge mask), SPMD across 8 NeuronCores.

Strategy (row-sharded scatter):
  - Core k owns rows [k*768, (k+1)*768) of the [6144, 6144] edge mask.
    Its block lives in SBUF as [128 partitions x 36864 f32] (partition p
    holds mask rows 6p..6p+5 of the block, row-major).
  - The host routes each scattered entry (both (r,c) and (c,r) of every
    edge) to (core, partition, window, slot). A window is a 768-f32
    column segment of one row: 48 windows per partition.
  - On device: ACT/DVE compute the gumbel-sigmoid edge values for the
    padded entry buffer; 48 gpsimd local_scatter calls compose the block
    (zeros + values fused). f32 exactness via the int16-pair bitcast
    trick (value f32 bits land as two adjacent int16 scatters).
  - 8 large HWDGE DMAs stream the block to HBM, pipelined with the
    scatters (chunk j = windows 6j..6j+5 = one SBUF tile).
"""

import sys
import types

for _p in ("/opt/trn_rl_repo", "/root/.axon_site"):
    if _p not in sys.path:
        sys.path.insert(0, _p)

# NTFF profile hook (used only when BASS_TRACE=1): the image's antenv lacks
# axon_hooks, so provide it via sys.modules before bass_utils imports it.
if "antenv.axon_hooks" not in sys.modules:
    _m = types.ModuleType("antenv.axon_hooks")

    def _get_hook():
        try:
            from trn_agent_boot.trn_boot import _ntff_profile_via_ctypes

            return _ntff_profile_via_ctypes("/opt/axon/libaxon_pjrt.so")
        except Exception:
            return None

    _m.get_axon_ntff_profile_hook = _get_hook
    _m.set_axon_ntff_profile_hook = lambda h: None
    sys.modules["antenv.axon_hooks"] = _m

import numpy as np

P = 6144          # num_patches
E = 262144        # number of edges
NCORES = 8
RPB = P // NCORES     # 768 rows per core block
RPP = RPB // 128      # 6 mask rows per partition
NCT = 8               # column tiles per row
WF = P // NCT         # 768 f32 per window
W = RPP * NCT         # 48 windows per partition
CHW = RPP             # windows per DMA chunk (6)
NCH = W // CHW        # 8 output DMA chunks
CHF = CHW * WF        # 4608 f32 per chunk per partition
TAU = 1.0
EPS = 1e-10

_BUILD_CACHE: dict[int, object] = {}
LAST_RESULTS = None   # BassKernelResults of the most recent run (for test.py)


def _build_program(K: int):
    """Build + finalize the SPMD Bass program for per-cell slot count K."""
    import concourse.bacc as bacc
    import concourse.mybir as mybir
    import concourse.tile as tile

    f32 = mybir.dt.float32
    i16 = mybir.dt.int16
    AF = mybir.ActivationFunctionType
    ALU = mybir.AluOpType
    LK = W * K
    NPF = P // 128  # 48 node values per partition

    nc = bacc.Bacc()
    # register EPS as a const AP so activation(bias=EPS) resolves
    _ct = nc.alloc_sbuf_tensor(f"const-f32-eps", [128, 1], f32)
    nc.gpsimd.memset(_ct.ap(), EPS)
    nc.const_aps.aps[(f32, EPS)] = _ct.ap()
    nc.all_engine_barrier()

    CK = CHW * K          # padded entries per partition per chunk
    SEG = 2 * CK          # int16 elems per segment (el / eu / ei)
    CPK = 3 * SEG         # packed int16 elems per chunk per partition

    pk = nc.declare_dram_parameter("pk", [128, NCH * CPK], i16, isOutput=False)
    nl = nc.declare_dram_parameter("nl", [P], f32, isOutput=False)
    nu = nc.declare_dram_parameter("nu", [P], f32, isOutput=False)
    eb = nc.declare_dram_parameter("edge_block", [RPB, P], f32, isOutput=True)
    nm = nc.declare_dram_parameter("node_mask", [P], f32, isOutput=True)

    with tile.TileContext(nc) as tc:
        with tc.tile_pool(name="sbuf", bufs=1) as pool:
            tpk = pool.tile([128, NCH * CPK], i16, tag="tpk")

            def load_chunk(j):
                s = slice(j * CPK, (j + 1) * CPK)
                nc.scalar.dma_start(tpk[:, s], pk[:, s])

            load_chunk(0)
            load_chunk(1)

            # ---- per-chunk: compute gumbel-sigmoid values, compose windows
            #      via local_scatter, stream chunk to HBM
            ebf = eb[:, :].rearrange("(p a) b -> p (a b)", p=128)  # [128, 36864]
            for j in range(NCH):
                if j + 2 < NCH:
                    load_chunk(j + 2)
                base = j * CPK
                elf = tpk[:, base : base + SEG].bitcast(f32)
                euf = tpk[:, base + SEG : base + 2 * SEG].bitcast(f32)
                # g = -ln(-ln(u + eps) + eps);  v = sigmoid((logit + g) / tau)
                nc.scalar.activation(euf, euf, AF.Ln, bias=EPS)
                # guard: ln(u+eps) must stay <= 0 so -ln(..)+eps > 0
                nc.vector.tensor_scalar_min(euf, euf, 0.0)
                nc.scalar.activation(euf, euf, AF.Ln, bias=EPS, scale=-1.0)
                nc.vector.tensor_tensor(elf, elf, euf, op=ALU.subtract)
                nc.scalar.activation(elf, elf, AF.Sigmoid, scale=1.0 / TAU)

                blk = pool.tile([128, CHF], f32, tag=f"blk{j}")
                for wi in range(CHW):
                    nc.gpsimd.local_scatter(
                        out_ap=blk[:, wi * WF : (wi + 1) * WF].bitcast(i16),
                        data_ap=tpk[:, base + wi * 2 * K : base + (wi + 1) * 2 * K],
                        idxs_ap=tpk[
                            :,
                            base + 2 * SEG + wi * 2 * K : base + 2 * SEG + (wi + 1) * 2 * K,
                        ],
                        channels=128,
                        num_elems=2 * WF,
                        num_idxs=2 * K,
                    )
                    # stream out per 2 windows: shorter pipeline tail and
                    # earlier DMA start than one DMA per 6-window chunk
                    if wi % 2 == 1:
                        lo = (j * CHW + wi - 1) * WF
                        hi = (j * CHW + wi + 1) * WF
                        nc.sync.dma_start(
                            ebf[:, lo:hi], blk[:, (wi - 1) * WF : (wi + 1) * WF]
                        )

            # ---- node mask (identical on every core; tiny — run at the end
            #      so it never delays the scatter pipeline)
            tnl = pool.tile([128, NPF], f32, tag="tnl")
            tnu = pool.tile([128, NPF], f32, tag="tnu")
            nc.scalar.dma_start(tnl[:], nl[:].rearrange("(a b) -> a b", a=128))
            nc.scalar.dma_start(tnu[:], nu[:].rearrange("(a b) -> a b", a=128))
            nc.scalar.activation(tnu[:], tnu[:], AF.Ln, bias=EPS)
            nc.vector.tensor_scalar_min(tnu[:], tnu[:], 0.0)
            nc.scalar.activation(tnu[:], tnu[:], AF.Ln, bias=EPS, scale=-1.0)
            nc.vector.tensor_tensor(tnl[:], tnl[:], tnu[:], op=ALU.subtract)
            nc.scalar.activation(tnl[:], tnl[:], AF.Sigmoid, scale=1.0 / TAU)
            nc.sync.dma_start(nm[:].rearrange("(a b) -> a b", a=128), tnl[:])

    nc.finalize()
    return nc


def _route_entries(rows: np.ndarray, cols: np.ndarray):
    """Route 2E scattered entries to (core, partition, window, slot).

    Returns (K, dest, order) where order indexes into the concatenated
    entry list (first E: (r,c), second E: (c,r)), dest is the flat slot
    index into the per-core padded buffers [NCORES, 128, W, K], and K the
    global max entries per (core, partition, window) cell.
    """
    rr = np.concatenate([rows, cols]).astype(np.int64)
    cc = np.concatenate([cols, rows]).astype(np.int64)

    core = rr // RPB
    lr = rr - core * RPB
    p = lr // RPP
    q = lr - p * RPP
    ct = cc // WF
    cpos = cc - ct * WF
    w = q * NCT + ct
    cell = (core * 128 + p) * W + w

    order = np.argsort(cell, kind="stable")
    cell_s = cell[order]
    # rank within equal-cell runs
    first = np.r_[0, np.flatnonzero(np.diff(cell_s)) + 1]
    counts = np.diff(np.r_[first, len(cell_s)])
    K = int(counts.max())
    slot = np.arange(len(cell_s), dtype=np.int64) - np.repeat(first, counts)
    dest = cell_s * K + slot
    return K, dest, order, cpos


def kernel(node_logits, edge_logits, u_node, u_edge, rows, cols):
    global LAST_RESULTS
    from concourse.bass_utils import run_bass_kernel_spmd

    node_logits = np.asarray(node_logits, np.float32)
    edge_logits = np.asarray(edge_logits, np.float32)
    u_node = np.asarray(u_node, np.float32)
    u_edge = np.asarray(u_edge, np.float32)
    rows = np.asarray(rows)
    cols = np.asarray(cols)

    K, dest, order, cpos = _route_entries(rows, cols)

    nc = _BUILD_CACHE.get(K)
    if nc is None:
        nc = _build_program(K)
        _BUILD_CACHE[K] = nc

    # padded per-core buffers (padding values never scattered: idx = -1;
    # u=0 padding is safe through the clamped log-log pipeline)
    ncell = NCORES * 128 * W
    el_pad = np.zeros(ncell * K, np.float32)
    eu_pad = np.zeros(ncell * K, np.float32)
    ei_pad = np.full(ncell * 2 * K, -1, np.int16)

    ee = np.concatenate([np.arange(E), np.arange(E)])[order]
    el_pad[dest] = edge_logits[ee]
    eu_pad[dest] = u_edge[ee]
    cpos_s = cpos[order]
    ei_pad[2 * dest] = (2 * cpos_s).astype(np.int16)
    ei_pad[2 * dest + 1] = (2 * cpos_s + 1).astype(np.int16)

    # pack [el | eu | ei] per chunk: [NC, 128, NCH, 3, SEG] int16
    SEG = 2 * CHW * K
    el16 = el_pad.view(np.int16).reshape(NCORES, 128, NCH, SEG)
    eu16 = eu_pad.view(np.int16).reshape(NCORES, 128, NCH, SEG)
    ei16 = ei_pad.reshape(NCORES, 128, NCH, SEG)
    pk = np.stack([el16, eu16, ei16], axis=3).reshape(NCORES, 128, NCH * 3 * SEG)

    in_maps = [
        {
            "pk": pk[c],
            "nl": node_logits,
            "nu": u_node,
        }
        for c in range(NCORES)
    ]

    res = run_bass_kernel_spmd(nc, in_maps, list(range(NCORES)))
    LAST_RESULTS = res

    edge_mask = np.concatenate(
        [res.results[c]["edge_block"] for c in range(NCORES)], axis=0
    )
    node_mask = res.results[0]["node_mask"]
    return node_mask, edge_mask
